# revision 30
# baseline (speedup 1.0000x reference)
"""Trainium2 Bass kernel for a grouped contrastive loss (v4).

Math (matches the reference):
    den[j] = sum_{a != j} exp((z_a . z_tj) / T),  z_a = [z_target; z_source]
    num[j] = mean_{s in group(j)} (z_s . z_tj) / T      (exact linearity)
    loss   = sum_j log(den[j]) - num[j]

The z_t x z_t part of den is symmetric: each unordered 128x128 tile pair is
computed ONCE. A circulant orientation of the 32-tile-column graph (tile u
feeds column t iff (t-u) mod 32 in 1..15, plus u = t-16 for t >= 16, plus
the diagonal) gives every column tile in-degree 16 or 17; cores take columns
{c, 15-c, 16+c, 31-c} so every core gets bands of 16,16,17,17 tiles — the
SAME program shape on all 8 cores, with the host permuting za per core.
This cuts ACT exp work 24% (32768 -> 24832 elems/lane per core) — ACT is
the only exp-capable engine and the bottleneck.

Pipeline per chunk (<=2048 cols): PE matmuls (fp16, f32 psum) -> ACT exp
(scale=1/T) to fp16 SBUF -> DVE fold-fold-reduce column sums (fp16
TensorTensor adds run in 2x DVE mode; plain f32 TensorReduce does not).
PSUM is one [128,2048] tag double-buffered (8 banks) so matmul refill
always overlaps the previous chunk's exp. Column sums of the transposed
band halves are tile ROW sums: Pool (GpSimd) partition_all_reduce over the
exp'd band ranges, shipped out as [1, W] strips — Pool is otherwise idle.
The diagonal is masked on-device by a DVE add of -2000*I onto the diag
psum range before exp (exp -> 0): no self-term cancellation on the host
and fp16 exp outputs cannot overflow. num is computed on the host.

The input is ONE tensor [-2000I | I | ones | zt | band0 | z_s | band1 |
band2 | band3] (z_s stored once; chunks gather scattered za ranges),
streamed as pieces split between the SP HWDGE queue and the Pool SWDGE
queue so neither the per-DMA issue cost nor the serial transfer chain gates
the exp stream. The last two chunks use ACT accum_out; all other results
are DMA'd out early, so the tail is one accumulator read + a 2-column DMA.
"""

import numpy as np

TEMPERATURE = 0.07
N = 4096
M = 4096
D = 128
G = 64
NCORES = 8
ZS = 4096
BW = [2048, 2048, 2176, 2176]        # band cols per slot (16,16,17,17 tiles)

# za column layout (single input tensor per core). zt slot0 sits right
# before band0 so one DMA piece delivers both lhsT and the first rhs.
NEG, EYE, ONE, ZT123, ZT0 = 0, 128, 256, 384, 768
B0, ZSB, B1, B2, B3 = 896, 2944, 7040, 9088, 11264
ZA_COLS = 13440
_BB = [B0, B1, B2, B3]

# chunks: (slot, [(za_lo, za_hi, is_diag)...]); all use one 2048-wide psum
# tag, double-buffered. Slot 0 ramps up small while the DMA stream fills.
# The diag tile is the LAST band tile: cols 1920:2048 for slots 0/1 (16
# tiles), cols 2048:2176 for slots 2/3 (17 tiles).
CHUNKS = [
    (0, [(B0, B0 + 512, 0)]),
    (0, [(B0 + 512, B0 + 1536, 0)]),
    (0, [(B0 + 1536, B0 + 1920, 0), (ZSB, ZSB + 1152, 0)]),
    (0, [(ZSB + 1152, ZSB + 3072, 0)]),
    (0, [(ZSB + 3072, ZSB + 4096, 0), (B0 + 1920, B0 + 2048, 1)]),

    (1, [(ZSB, ZSB + 2048, 0)]),
    (1, [(ZSB + 2048, ZSB + 4096, 0)]),
    (1, [(B1, B1 + 1920, 0), (B1 + 1920, B1 + 2048, 1)]),

    # ALL bands run before the z_s rereads so every Pool all-reduce and
    # res2 strip DMA hides mid-stream instead of trailing the kernel tail;
    # the last two chunks are pure z_s with ACT accum_out
    (2, [(B2, B2 + 1024, 0)]),
    (2, [(B2 + 1024, B2 + 2048, 0), (ZSB, ZSB + 1024, 0)]),
    (3, [(B3, B3 + 2048, 0)]),
    (3, [(ZSB, ZSB + 1920, 0), (B3 + 2048, B3 + 2176, 1)]),

    (2, [(ZSB + 1024, ZSB + 2944, 0), (B2 + 2048, B2 + 2176, 1)]),
    (2, [(ZSB + 2944, ZSB + 4096, 0)]),
    (3, [(ZSB + 1920, ZSB + 3968, 0)]),
    (3, [(ZSB + 3968, ZSB + 4096, 0)]),
]
NCHUNK = len(CHUNKS)                 # 16
ACCUM_CHUNKS = {14: 14, 15: 15}      # chunk -> res col (reduces use cols 0..13)
RES_COLS = 16
RS2_COLS = 8448                      # [b0 2048 | b1 2048 | b2 2176 | b3 2176]
_RS2_BASE = [0, 2048, 4096, 6272]

# DMA pieces per queue, in issue order (SP = HWDGE, Pool = SWDGE)
DMA_SP = [(ZT0, B0 + 512), (B0 + 1536, B0 + 2048), (ZSB, ZSB + 1152),
          (ZSB + 1152, ZSB + 3072), (ZSB + 3072, ZSB + 4096),
          (B1, B1 + 2048), (B3, B3 + 2176)]
DMA_POOL = [(B0 + 512, B0 + 1536), (0, ZT123), (ZT123, ZT0),
            (B2, B2 + 2176)]


def tset(c):
    return [c, 15 - c, 16 + c, 31 - c]


def band(t):
    nb = []
    if t >= 16:
        nb.append(t - 16)
    nb += [(t - 15 + m) % 32 for m in range(15)]
    return nb + [t]                  # diagonal tile last


def _band_ranges(k, pieces):
    """[(chunk_off, width, res2_col)] for non-diag band cols in this chunk.
    Diag tiles are skipped: the host never reads their rowsums."""
    out = []
    off = 0
    b = _BB[k]
    for lo, hi, d in pieces:
        if not d and b <= lo and hi <= b + BW[k]:
            out.append((off, hi - lo, _RS2_BASE[k] + lo - b))
        off += hi - lo
    return out


_CACHE = {}


def _build_bass():
    import concourse.mybir as mybir
    from concourse import bacc
    from concourse import bass_isa
    from concourse.tile import TileContext

    f32 = mybir.dt.float32
    f16 = mybir.dt.float16
    Exp = mybir.ActivationFunctionType.Exp

    nc = bacc.Bacc("TRN2", num_devices=NCORES)
    za = nc.dram_tensor("za", [D, ZA_COLS], f16, kind="ExternalInput")
    res = nc.dram_tensor("res", [128, RES_COLS], f32, kind="ExternalOutput")
    res2 = nc.dram_tensor("res2", [1, RS2_COLS], f32, kind="ExternalOutput")

    with TileContext(nc) as tc:
        with (
            tc.tile_pool(name="persist", bufs=1) as persist,
            tc.tile_pool(name="scr", bufs=5) as scr_pool,
            tc.tile_pool(name="folds", bufs=2) as fold_pool,
            tc.tile_pool(name="prout", bufs=2) as pr_pool,
            tc.tile_pool(name="psmain", bufs=2, space="PSUM") as psum_pool,
        ):
            za_tile = persist.tile([128, ZA_COLS], f16, tag="za")
            for lo, hi in DMA_SP:
                nc.sync.dma_start(out=za_tile[:, lo:hi], in_=za[:, lo:hi])
            for lo, hi in DMA_POOL:
                nc.gpsimd.dma_start(out=za_tile[:, lo:hi], in_=za[:, lo:hi])
            res_tile = persist.tile([128, RES_COLS], f32, tag="res")

            neg_i = za_tile[:, NEG:NEG + 128]

            pending_fold = None
            for ci, (k, pieces) in enumerate(CHUNKS):
                zk = ZT0 if k == 0 else ZT123 + (k - 1) * 128
                lhsT = za_tile[:, zk:zk + 128]
                w = sum(hi - lo for lo, hi, _ in pieces)
                ps = psum_pool.tile([128, 2048], f32, tag="ps")
                o = 0
                for lo, hi, is_diag in pieces:
                    p = lo
                    while p < hi:
                        # a matmul output must stay inside one 512-f32 psum
                        # bank: split at the 512-grid of the chunk offset
                        pe = min(p + 512 - o % 512, hi)
                        nc.tensor.matmul(
                            ps[:, o:o + pe - p], lhsT, za_tile[:, p:pe],
                            start=True, stop=True,
                        )
                        if is_diag:
                            # mask the self-similarity diagonal before exp
                            # (DVE add is deterministically ordered between
                            # the matmul write and the ACT read)
                            nc.vector.tensor_add(
                                out=ps[:, o:o + pe - p],
                                in0=ps[:, o:o + pe - p], in1=neg_i)
                        o += pe - p
                        p = pe
                scr = scr_pool.tile([128, 2048], f16, tag="scr")
                acc_col = ACCUM_CHUNKS.get(ci)
                nc.scalar.activation(
                    out=scr[:, 0:w], in_=ps[:, 0:w],
                    func=Exp, scale=1.0 / TEMPERATURE,
                    accum_out=None if acc_col is None
                    else res_tile[:, acc_col:acc_col + 1],
                )

                def emit_folds(ci_, w_, scr_):
                    # fp16 fold-fold-reduce: TT adds run in 2x DVE mode
                    # (tensor_tensor_reduce would be one op but hits an
                    # INTERNAL error on this hardware path)
                    h2, q = w_ // 2, w_ // 4
                    f1 = fold_pool.tile([128, 1024], f16, tag="f1")
                    f2 = fold_pool.tile([128, 512], f16, tag="f2")
                    nc.vector.tensor_add(
                        out=f1[:, :h2], in0=scr_[:, 0:h2], in1=scr_[:, h2:w_])
                    nc.vector.tensor_add(
                        out=f2[:, :q], in0=f1[:, 0:q], in1=f1[:, q:h2])
                    nc.vector.tensor_reduce(
                        out=res_tile[:, ci_:ci_ + 1], in_=f2[:, :q],
                        axis=mybir.AxisListType.X, op=mybir.AluOpType.add)

                # DVE folds are deferred by one chunk so the next chunk's
                # diag-mask add sits ahead of them in DVE's in-order queue
                # (the mask gates ACT; folds have a chunk of slack)
                if pending_fold is not None:
                    emit_folds(*pending_fold)
                pending_fold = (ci, w, scr) if acc_col is None else None
                # transposed-half contributions: partition sums of the band
                # ranges on Pool, shipped as [1, W] strips
                for off, bw_, col in _band_ranges(k, pieces):
                    pr = pr_pool.tile([128, 2048], f32, tag="pr")
                    nc.gpsimd.partition_all_reduce(
                        pr[:, 0:bw_], scr[:, off:off + bw_],
                        channels=128, reduce_op=bass_isa.ReduceOp.add)
                    nc.sync.dma_start(
                        out=res2[0:1, col:col + bw_], in_=pr[0:1, 0:bw_])
                if ci == 14:
                    # all 14 reduce cols are final (ci13's folds flushed at
                    # the top of this block): ship them while ci14/15 stream
                    nc.sync.dma_start(out=res[:, 0:14], in_=res_tile[:, 0:14])
            assert pending_fold is None
            nc.sync.dma_start(out=res[:, 14:16], in_=res_tile[:, 14:16])
    nc.compile()
    return nc


def _get_nc():
    if "nc" not in _CACHE:
        _CACHE["nc"] = _build_bass()
    return _CACHE["nc"]


def make_inmaps(z_source, z_target):
    """Host-side sharding: per-core za = [-2000I | I | ones | zt | bands/zs]."""
    zs = np.ascontiguousarray(z_source, dtype=np.float32)
    zt = np.ascontiguousarray(z_target, dtype=np.float32)
    za16 = np.concatenate([zt, zs], axis=0).astype(np.float16)   # [8192, D]
    za16T = np.ascontiguousarray(za16.T)                         # [D, 8192]
    eye = np.eye(128, dtype=np.float16)
    in_maps = []
    for c in range(NCORES):
        zac = np.zeros((D, ZA_COLS), np.float16)
        zac[:, NEG:NEG + 128] = -2000.0 * eye
        zac[:, EYE:EYE + 128] = eye
        zac[:, ONE] = 1.0
        zac[:, ZSB:ZSB + ZS] = za16T[:, 4096:8192]
        for k, t in enumerate(tset(c)):
            zk = ZT0 if k == 0 else ZT123 + (k - 1) * 128
            zac[:, zk:zk + 128] = za16T[:, 128 * t:128 * t + 128]
            cols = np.concatenate(
                [np.arange(128 * u, 128 * u + 128) for u in band(t)])
            zac[:, _BB[k]:_BB[k] + BW[k]] = za16T[:, cols]
        in_maps.append({"za": zac})
    return in_maps


def kernel(z_source, z_target, seg_source, seg_target):
    from concourse.bass_utils import run_bass_kernel_spmd

    zs = np.ascontiguousarray(z_source, dtype=np.float32)
    zt = np.ascontiguousarray(z_target, dtype=np.float32)
    seg_s = np.asarray(seg_source).astype(np.int64)
    seg_t = np.asarray(seg_target).astype(np.int64)

    in_maps = make_inmaps(zs, zt)
    nc = _get_nc()
    out = run_bass_kernel_spmd(nc, in_maps, core_ids=list(range(NCORES)))
    results = out.results

    slot_cols = [[] for _ in range(4)]
    for ci, (k, _p) in enumerate(CHUNKS):
        slot_cols[k].append(ACCUM_CHUNKS.get(ci, ci))

    den = np.zeros(M, np.float64)
    for c in range(NCORES):
        r = results[c]["res"].astype(np.float64)     # [128, 16]
        r2 = results[c]["res2"].astype(np.float64)[0]  # [8448]
        for k, t in enumerate(tset(c)):
            den[128 * t:128 * t + 128] += r[:, slot_cols[k]].sum(axis=1)
            bt = band(t)
            for i, u in enumerate(bt[:-1]):          # skip the diag tile
                den[128 * u:128 * u + 128] += \
                    r2[_RS2_BASE[k] + 128 * i:_RS2_BASE[k] + 128 * i + 128]

    # num on host in f64 (exact group-mean linearity)
    counts = np.bincount(seg_s, minlength=G).astype(np.float64)
    S = np.zeros((G, D), np.float64)
    np.add.at(S, seg_s, zs.astype(np.float64))
    v = S[seg_t] / (counts[seg_t] * TEMPERATURE)[:, None]
    num = np.einsum("md,md->m", v, zt.astype(np.float64))

    loss = np.sum(np.log(den)) - np.sum(num)
    return np.asarray(loss, dtype=np.float32)
